# revision 1
# baseline (speedup 1.0000x reference)
"""RMT memory kernel for 8 Trainium2 NeuronCores.

Core c = (batch b=c//4, token-quarter t=c%4); 256 tokens/segment/core.
Read-attention, gates, h token-local. Write-attention numerators/denoms
AllReduced per 4-core batch group (AR#1). Memory update replicated
post-AR; next-segment k_r/v_r projections K-sharded over the group and
summed in AR#2, which also redistributes the full memory state.

Everything on device is D-major ("transposed"); host pre-transposes
inputs and re-transposes outputs. All matmuls bf16 (host-cast weights);
the only precision-critical op, h = seg + gate*o, adds seg in fp32.
"""

import sys

sys.path.insert(0, "/opt/trn_rl_repo")

import numpy as np
import ml_dtypes

D = 2560
H = 8
HD = 320
MEM = 128
NSEG = 4
TOK = 256
NT = D // 128       # 20
SL = 640            # per-core D slice
GROUPS = [[0, 1, 2, 3], [4, 5, 6, 7]]
SCALE = 1.0 / float(np.sqrt(HD))
BF16 = ml_dtypes.bfloat16

_CACHE = {}


def _pieces(start, size):
    """Split [start, start+size) at 128-boundaries."""
    out, p, end = [], start, start + size
    while p < end:
        nxt = min(end, (p // 128 + 1) * 128)
        out.append((p, nxt - p))
        p = nxt
    return out


def _build(debug=False):
    import concourse.bass as bass
    import concourse.bacc as bacc
    import concourse.mybir as mybir
    import concourse.tile as tile

    f32 = mybir.dt.float32
    bf16 = mybir.dt.bfloat16
    AF = mybir.ActivationFunctionType

    nc = bacc.Bacc("TRN2", target_bir_lowering=False, debug=False, num_devices=8)

    xt32 = nc.dram_tensor("xt32", [D, NSEG * TOK], f32, kind="ExternalInput")
    xtb = nc.dram_tensor("xtb", [D, NSEG * TOK], bf16, kind="ExternalInput")
    wqr = nc.dram_tensor("wqr", [D, D], bf16, kind="ExternalInput")
    wgr = nc.dram_tensor("wgr", [D, D], bf16, kind="ExternalInput")
    wqw = nc.dram_tensor("wqw", [D, D], bf16, kind="ExternalInput")
    wor = nc.dram_tensor("wor", [D, D], bf16, kind="ExternalInput")
    wkw = nc.dram_tensor("wkw", [D, D], bf16, kind="ExternalInput")
    wvw = nc.dram_tensor("wvw", [D, D], bf16, kind="ExternalInput")
    wow = nc.dram_tensor("wow", [D, D], bf16, kind="ExternalInput")
    wgw_sl = nc.dram_tensor("wgw_sl", [2 * D, SL], bf16, kind="ExternalInput")
    wkr_sl = nc.dram_tensor("wkr_sl", [SL, D], bf16, kind="ExternalInput")
    wvr_sl = nc.dram_tensor("wvr_sl", [SL, D], bf16, kind="ExternalInput")
    bgr = nc.dram_tensor("bgr", [D, 1], f32, kind="ExternalInput")
    bgw_sl = nc.dram_tensor("bgw_sl", [SL, 1], f32, kind="ExternalInput")
    mqt = nc.dram_tensor("mqt", [D, MEM], bf16, kind="ExternalInput")
    m0t = nc.dram_tensor("m0t", [D, MEM], bf16, kind="ExternalInput")

    ht = nc.dram_tensor("ht", [D, NSEG * TOK], f32, kind="ExternalOutput")
    dbg = nc.dram_tensor("dbg", [128, 53504], bf16, kind="ExternalOutput") \
        if debug else None

    with tile.TileContext(nc) as tc:
        with (
            tc.tile_pool(name="dram", bufs=1, space="DRAM") as dram,
            tc.tile_pool(name="res", bufs=1) as res,
            tc.tile_pool(name="wrk", bufs=1) as wrk,
            tc.tile_pool(name="stream", bufs=1) as stream,
            tc.tile_pool(name="ps", bufs=8, space="PSUM") as psp,
        ):
            # ---------------- DRAM scratch ----------------
            qr_sp = dram.tile([128, NT * NSEG * TOK], bf16)  # seg-major
            gt_sp = dram.tile([128, NT * NSEG * TOK], bf16)
            b1_in = dram.tile([129, NT * 128], bf16)
            b1_out = dram.tile([129, NT * 128], bf16)
            b2_in = dram.tile([384, NT * 128], bf16)
            b2_out = dram.tile([384, NT * 128], bf16)

            def psum(n=512):
                t = psp.tile([128, 512], f32, tag="ps", bufs=8)
                return t[:, :n] if n != 512 else t[:]

            _dbg_off = [0]

            def dump(ap, cols):
                if dbg is None:
                    return
                o = _dbg_off[0]
                nc.gpsimd.dma_start(dbg[:, o:o + cols], ap)
                _dbg_off[0] = o + cols

            # ---------------- residents ----------------
            xtb_sb = res.tile([128, NT * NSEG * TOK], bf16)
            qwT = res.tile([128, NT * 128], bf16)
            mem = res.tile([128, NT * 128], bf16)
            krT = res.tile([128, NT * 128], bf16)
            vr = res.tile([128, NT * 128], bf16)
            ones_bf = res.tile([128, 1], bf16)
            nc.vector.memset(ones_bf[:], 1.0)

            for k in range(NT):
                nc.gpsimd.dma_start(
                    xtb_sb[:, k * NSEG * TOK:(k + 1) * NSEG * TOK],
                    xtb[k * 128:(k + 1) * 128, :])
                nc.gpsimd.dma_start(mem[:, k * 128:(k + 1) * 128],
                                    m0t[k * 128:(k + 1) * 128, :])

            pid = nc.vector.partition_id()
            toff = nc.snap((pid % 4) * SL, donate=False, min_val=0,
                           max_val=3 * SL)

            # ---------------- helpers ----------------
            def gemm(w_dram, rhs_get, n, evac, kt=NT, mt=NT, mg=4, tag="w",
                     eng=None):
                """out^T[m, :n] += sum_k w[k, m-block].T @ rhs_get(k).

                Streams one [128, mg*128] weight tile per k (sequential use,
                bufs=3 prefetch); mg psum tiles stay resident across the
                k-accumulation."""
                eng = eng or nc.sync
                for m0 in range(0, mt, mg):
                    g = min(mg, mt - m0)
                    pss = [psum(n) for _ in range(g)]
                    for k in range(kt):
                        wt = stream.tile([128, g * 128], bf16, tag=tag, bufs=3)
                        eng.dma_start(
                            wt[:], w_dram[k * 128:(k + 1) * 128,
                                          m0 * 128:(m0 + g) * 128])
                        r = rhs_get(k)
                        for mi in range(g):
                            nc.tensor.matmul(
                                pss[mi], wt[:, mi * 128:(mi + 1) * 128], r,
                                start=(k == 0), stop=(k == kt - 1))
                    for mi in range(g):
                        evac(m0 + mi, pss[mi])

            # ---------------- q_w^T (one-time) ----------------
            def mqt_rhs(k):
                t = stream.tile([128, 128], bf16, tag="rstream", bufs=3)
                nc.gpsimd.dma_start(t[:], mqt[k * 128:(k + 1) * 128, :])
                return t[:]
            gemm(wqw, mqt_rhs, 128,
                 lambda m, ps: nc.scalar.copy(qwT[:, m * 128:(m + 1) * 128], ps),
                 mg=4, tag="w")

            # ---------------- k/v partials + AR#2 ----------------
            def kv_partials_and_ar2(memsl_src, memstage_or_none):
                """memsl_src: [128, 5*128] slice rows of (updated) mem^T."""
                pk = wrk.tile([128, NT * 128], bf16, tag="pkpv", bufs=2)
                pv = wrk.tile([128, NT * 128], bf16, tag="pkpv", bufs=2)
                gemm(wkr_sl,
                     lambda kk: memsl_src[:, kk * 128:(kk + 1) * 128], 128,
                     lambda m, ps: nc.scalar.copy(
                         pk[:, m * 128:(m + 1) * 128], ps),
                     kt=5, mt=NT, mg=4, tag="w")
                for ch in range(5):
                    ps = psum(512)
                    for kk in range(5):
                        wt = stream.tile([128, 512], bf16, tag="w", bufs=3)
                        nc.sync.dma_start(
                            wt[:], wvr_sl[kk * 128:(kk + 1) * 128,
                                          ch * 512:(ch + 1) * 512])
                        nc.tensor.matmul(
                            ps, memsl_src[:, kk * 128:(kk + 1) * 128],
                            wt[:], start=(kk == 0), stop=(kk == 4))
                    nc.scalar.copy(pv[:, ch * 512:(ch + 1) * 512], ps)
                nc.gpsimd.dma_start(b2_in[0:128, :], pk[:])
                nc.gpsimd.dma_start(b2_in[128:256, :], pv[:])
                if memstage_or_none is not None:
                    nc.gpsimd.dma_start(b2_in[256:384, :], memstage_or_none[:])
                nc.gpsimd.collective_compute(
                    "AllReduce", mybir.AluOpType.add,
                    ins=[b2_in.opt()], outs=[b2_out.opt()],
                    replica_groups=GROUPS)
                nc.gpsimd.dma_start(krT[:], b2_out[0:128, :])
                nc.gpsimd.dma_start(vr[:], b2_out[128:256, :])
                if memstage_or_none is not None:
                    nc.gpsimd.dma_start(mem[:], b2_out[256:384, :])

            # seg-0 bootstrap: k_r/v_r from initial memory
            memsl0 = wrk.tile([128, 5 * 128], bf16, tag="memsl")
            nc.vector.tensor_copy(memsl0[:], mem[:, bass.ds(toff, SL)])
            kv_partials_and_ar2(memsl0, None)

            # ---------------- phase-1: q_r^T, gate^T ----------------
            def phase1(w_dram, sp, bias_dram, tag):
                def evac(m, chunk, ps):
                    if bias_dram is None:
                        tmp = wrk.tile([128, 512], bf16, tag="p1ev", bufs=2)
                        nc.scalar.copy(tmp[:], ps)
                    else:
                        bt = wrk.tile([128, 1], f32, tag="p1b", bufs=2)
                        nc.scalar.dma_start(
                            bt[:], bias_dram[m * 128:(m + 1) * 128, :])
                        tmp = wrk.tile([128, 512], bf16, tag="p1ev", bufs=2)
                        nc.scalar.activation(tmp[:], ps, AF.Sigmoid, bias=bt[:])
                    for sh in range(2):  # two segments per 512 chunk
                        s = chunk * 2 + sh
                        dst_col = s * NT * TOK + m * TOK
                        nc.scalar.dma_start(
                            sp[:, dst_col:dst_col + TOK],
                            tmp[:, sh * TOK:(sh + 1) * TOK])
                for m0 in range(0, NT, 2):
                    pss = [[psum(512) for _ in range(2)] for _ in range(2)]
                    for k in range(NT):
                        wt = stream.tile([128, 256], bf16, tag=tag, bufs=3)
                        nc.scalar.dma_start(
                            wt[:], w_dram[k * 128:(k + 1) * 128,
                                          m0 * 128:(m0 + 2) * 128])
                        for mi in range(2):
                            for chunk in range(2):
                                nc.tensor.matmul(
                                    pss[mi][chunk],
                                    wt[:, mi * 128:(mi + 1) * 128],
                                    xtb_sb[:, k * NSEG * TOK + chunk * 512:
                                           k * NSEG * TOK + (chunk + 1) * 512],
                                    start=(k == 0), stop=(k == NT - 1))
                    for mi in range(2):
                        for chunk in range(2):
                            evac(m0 + mi, chunk, pss[mi][chunk])

            phase1(wqr, qr_sp, None, "p1q")
            phase1(wgr, gt_sp, bgr, "p1g")

            # ---------------- segment chain ----------------
            for s in range(NSEG):
                # -- reload q_r^T / gate^T for this segment --
                qrT_sb = wrk.tile([128, NT * TOK], bf16, tag="qrT")
                gtT_sb = wrk.tile([128, NT * TOK], bf16, tag="gtT")
                nc.scalar.dma_start(
                    qrT_sb[:], qr_sp[:, s * NT * TOK:(s + 1) * NT * TOK])
                nc.scalar.dma_start(
                    gtT_sb[:], gt_sp[:, s * NT * TOK:(s + 1) * NT * TOK])

                # -- read attention --
                E_sb = wrk.tile([128, H * TOK], bf16, tag="E")
                for h in range(H):
                    ps = psum(TOK)
                    pcs = _pieces(h * HD, HD)
                    for i, (off, sz) in enumerate(pcs):
                        g, o = off // 128, off % 128
                        nc.tensor.matmul(
                            ps, krT[o:o + sz, g * 128:(g + 1) * 128],
                            qrT_sb[o:o + sz, g * TOK:(g + 1) * TOK],
                            start=(i == 0), stop=(i == len(pcs) - 1))
                    nc.scalar.activation(E_sb[:, h * TOK:(h + 1) * TOK], ps,
                                         AF.Exp, scale=SCALE)
                # read denominators: [1, 256] per head, 4 heads per psum tile
                dpack = wrk.tile([128, H * TOK], f32, tag="dpack")
                dens = dpack[0:1, :]
                for hp in range(H // 2):
                    ps = psum(512)
                    for hh in range(2):
                        h = hp * 2 + hh
                        nc.tensor.matmul(
                            ps[0:1, hh * TOK:(hh + 1) * TOK], ones_bf[:],
                            E_sb[:, h * TOK:(h + 1) * TOK],
                            start=True, stop=True)
                    nc.vector.tensor_copy(
                        dens[:, hp * 512:(hp + 1) * 512], ps[0:1, :])
                recip = wrk.tile([1, H * TOK], f32, tag="recip")
                nc.vector.reciprocal(recip[:], dens[:])
                rrep = {}
                for h in range(H):
                    rrep[h] = wrk.tile([128, TOK], f32, tag="rrep", bufs=2, name="rrep_h")
                    nc.gpsimd.partition_broadcast(
                        rrep[h][:], recip[:, h * TOK:(h + 1) * TOK])

                # ctx (normalized at evacuation)
                ctx_sb = wrk.tile([128, NT * TOK], bf16, tag="ctx")
                ctx_ps = {}
                for h in range(H):
                    for off, sz in _pieces(h * HD, HD):
                        g, o = off // 128, off % 128
                        if g not in ctx_ps:
                            ctx_ps[g] = (psum(TOK), [])
                        ps, evl = ctx_ps[g]
                        nc.tensor.matmul(
                            ps[o:o + sz, :], vr[:, off:off + sz],
                            E_sb[:, h * TOK:(h + 1) * TOK],
                            start=True, stop=True)
                        evl.append((h, o, sz))
                        if o + sz == 128 or (h == H - 1 and off + sz == D):
                            for (hh, oo, ss) in evl:
                                nc.vector.tensor_mul(
                                    ctx_sb[oo:oo + ss, g * TOK:(g + 1) * TOK],
                                    ps[oo:oo + ss, :],
                                    rrep[hh][oo:oo + ss, :])
                            ctx_ps[g] = None

                if s == 0:
                    dump(krT[:], NT * 128)
                    dump(vr[:], NT * 128)
                    dump(qrT_sb[:], NT * TOK)
                    dump(gtT_sb[:], NT * TOK)
                    dump(E_sb[:], H * TOK)
                    dump(ctx_sb[:], NT * TOK)

                # -- o^T = wo_r.T @ ctx ; h = seg + gate*o --
                hbf = wrk.tile([128, NT * TOK], bf16, tag="hbf")

                def evac_o(m, ps):
                    seg32 = wrk.tile([128, TOK], f32, tag="seg32", bufs=2)
                    nc.scalar.dma_start(
                        seg32[:], xt32[m * 128:(m + 1) * 128,
                                       s * TOK:(s + 1) * TOK])
                    tmp = wrk.tile([128, TOK], f32, tag="otmp", bufs=2)
                    nc.vector.tensor_mul(
                        tmp[:], ps, gtT_sb[:, m * TOK:(m + 1) * TOK])
                    h32 = wrk.tile([128, TOK], f32, tag="h32", bufs=2)
                    nc.vector.tensor_add(h32[:], tmp[:], seg32[:])
                    nc.scalar.dma_start(
                        ht[m * 128:(m + 1) * 128, s * TOK:(s + 1) * TOK],
                        h32[:])
                    nc.vector.tensor_copy(hbf[:, m * TOK:(m + 1) * TOK],
                                          h32[:])

                gemm(wor, lambda k: ctx_sb[:, k * TOK:(k + 1) * TOK], TOK,
                     evac_o, mg=4, tag="w")

                # -- k_w^T, v_w --
                kwT = wrk.tile([128, NT * TOK], bf16, tag="kwT")
                gemm(wkw, lambda k: hbf[:, k * TOK:(k + 1) * TOK], TOK,
                     lambda m, ps: nc.scalar.copy(
                         kwT[:, m * TOK:(m + 1) * TOK], ps),
                     mg=4, tag="w", eng=nc.scalar)
                vw = wrk.tile([128, 2 * D], bf16, tag="vw")
                for ch in range(5):
                    pss = [psum(512), psum(512)]
                    for k in range(NT):
                        wt = stream.tile([128, 512], bf16, tag="w", bufs=3)
                        nc.scalar.dma_start(
                            wt[:], wvw[k * 128:(k + 1) * 128,
                                       ch * 512:(ch + 1) * 512])
                        for tt in range(2):
                            nc.tensor.matmul(
                                pss[tt],
                                hbf[:, k * TOK + tt * 128:k * TOK + tt * 128 + 128],
                                wt[:], start=(k == 0), stop=(k == NT - 1))
                    for tt in range(2):
                        nc.scalar.copy(
                            vw[:, tt * D + ch * 512:tt * D + (ch + 1) * 512],
                            pss[tt])

                if s == 0:
                    dump(hbf[:], NT * TOK)
                    dump(kwT[:], NT * TOK)
                    dump(vw[:], 2 * D)

                # -- write attention --
                ewT = wrk.tile([128, H * 2 * 128], bf16, tag="ewT")
                for h in range(H):
                    for tt in range(2):
                        ps = psum(128)
                        pcs = _pieces(h * HD, HD)
                        for i, (off, sz) in enumerate(pcs):
                            g, o = off // 128, off % 128
                            nc.tensor.matmul(
                                ps,
                                kwT[o:o + sz,
                                    g * TOK + tt * 128:g * TOK + tt * 128 + 128],
                                qwT[o:o + sz, g * 128:(g + 1) * 128],
                                start=(i == 0), stop=(i == len(pcs) - 1))
                        nc.scalar.activation(
                            ewT[:, (h * 2 + tt) * 128:(h * 2 + tt + 1) * 128],
                            ps, AF.Exp, scale=SCALE)
                densw = dpack[64:65, 0:H * 128]
                for hp in range(H // 4):
                    ps = psum(512)
                    for hh in range(4):
                        h = hp * 4 + hh
                        for tt in range(2):
                            nc.tensor.matmul(
                                ps[0:1, hh * 128:(hh + 1) * 128], ones_bf[:],
                                ewT[:, (h * 2 + tt) * 128:(h * 2 + tt + 1) * 128],
                                start=(tt == 0), stop=(tt == 1))
                    nc.vector.tensor_copy(
                        densw[:, hp * 512:(hp + 1) * 512], ps[0:1, :])
                dbpack = wrk.tile([64, H * 128], bf16, tag="dbpack")
                densw_bf = dbpack[0:1, :]
                nc.vector.tensor_copy(densw_bf[:], densw[:])

                attn = wrk.tile([128, NT * 128], bf16, tag="attn", bufs=2)
                att_ps = {}
                for h in range(H):
                    for off, sz in _pieces(h * HD, HD):
                        g, o = off // 128, off % 128
                        if g not in att_ps:
                            att_ps[g] = (psum(128), [])
                        ps, evl = att_ps[g]
                        for tt in range(2):
                            nc.tensor.matmul(
                                ps[o:o + sz, :],
                                vw[:, tt * D + off:tt * D + off + sz],
                                ewT[:, (h * 2 + tt) * 128:(h * 2 + tt + 1) * 128],
                                start=(tt == 0), stop=(tt == 1))
                        evl.append((h, o, sz))
                        if o + sz == 128 or (h == H - 1 and off + sz == D):
                            for (hh, oo, ss) in evl:
                                nc.scalar.copy(
                                    attn[oo:oo + ss, g * 128:(g + 1) * 128],
                                    ps[oo:oo + ss, :])
                            att_ps[g] = None

                if s == 0:
                    dump(ewT[:], H * 2 * 128)
                    dump(attn[:], NT * 128)

                # -- AR#1 --
                nc.gpsimd.dma_start(b1_in[0:128, :], attn[:])
                nc.gpsimd.dma_start(b1_in[128:129, 0:H * 128], densw_bf[:])
                nc.gpsimd.collective_compute(
                    "AllReduce", mybir.AluOpType.add,
                    ins=[b1_in.opt()], outs=[b1_out.opt()],
                    replica_groups=GROUPS)
                attnS = wrk.tile([128, NT * 128], bf16, tag="attn", bufs=2)
                denswS = dbpack[32:33, :]
                nc.gpsimd.dma_start(attnS[:], b1_out[0:128, :])
                nc.gpsimd.dma_start(denswS[:], b1_out[128:129, 0:H * 128])
                denswS32 = dpack[96:97, 0:H * 128]
                nc.vector.tensor_copy(denswS32[:], denswS[:])
                recw = wrk.tile([1, H * 128], f32, tag="recw")
                nc.vector.reciprocal(recw[:], denswS32[:])
                rwrep = {}
                for h in range(H):
                    rwrep[h] = wrk.tile([128, 128], f32, tag="rwrep", bufs=2, name="rwrep_h")
                    nc.gpsimd.partition_broadcast(
                        rwrep[h][:], recw[:, h * 128:(h + 1) * 128])
                attnN = wrk.tile([128, NT * 128], bf16, tag="attn", bufs=2)
                for h in range(H):
                    for off, sz in _pieces(h * HD, HD):
                        g, o = off // 128, off % 128
                        nc.vector.tensor_mul(
                            attnN[o:o + sz, g * 128:(g + 1) * 128],
                            attnS[o:o + sz, g * 128:(g + 1) * 128],
                            rwrep[h][o:o + sz, :])

                if s == 0:
                    dump(attnN[:], NT * 128)

                # -- new_mem^T (full, replicated) --
                new_sb = wrk.tile([128, NT * 128], bf16, tag="new")
                gemm(wow, lambda k: attnN[:, k * 128:(k + 1) * 128], 128,
                     lambda m, ps: nc.scalar.copy(
                         new_sb[:, m * 128:(m + 1) * 128], ps),
                     mg=4, tag="w", eng=nc.scalar)

                # -- gate (slice-local) + memory update --
                g_ps = [psum(128) for _ in range(5)]
                for k in range(2 * NT):
                    wt = stream.tile([128, SL], bf16, tag="wgw", bufs=3)
                    nc.sync.dma_start(
                        wt[:], wgw_sl[k * 128:(k + 1) * 128, :])
                    rhs = (mem[:, k * 128:(k + 1) * 128] if k < NT
                           else new_sb[:, (k - NT) * 128:(k - NT + 1) * 128])
                    for m in range(5):
                        nc.tensor.matmul(
                            g_ps[m], wt[:, m * 128:(m + 1) * 128], rhs,
                            start=(k == 0), stop=(k == 2 * NT - 1))
                gsl = wrk.tile([128, 5 * 128], f32, tag="gsl")
                for m in range(5):
                    bt = wrk.tile([128, 1], f32, tag="bgw_t", bufs=2)
                    nc.scalar.dma_start(bt[:], bgw_sl[m * 128:(m + 1) * 128, :])
                    nc.scalar.activation(gsl[:, m * 128:(m + 1) * 128],
                                         g_ps[m], AF.Sigmoid, bias=bt[:])

                new_sl = wrk.tile([128, 5 * 128], bf16, tag="new_sl")
                nc.vector.tensor_copy(new_sl[:], new_sb[:, bass.ds(toff, SL)])
                memsl = wrk.tile([128, 5 * 128], bf16, tag="memsl")
                nc.vector.tensor_copy(memsl[:], mem[:, bass.ds(toff, SL)])
                dlt = wrk.tile([128, 5 * 128], f32, tag="dlt")
                nc.vector.tensor_sub(dlt[:], new_sl[:], memsl[:])
                nc.vector.tensor_mul(dlt[:], gsl[:], dlt[:])
                memsl_n = wrk.tile([128, 5 * 128], bf16, tag="memsl_n")
                nc.vector.tensor_add(memsl_n[:], memsl[:], dlt[:])

                if s == 0 and dbg is not None:
                    dump(new_sb[:], NT * 128)
                    gslb = wrk.tile([128, 5 * 128], bf16, tag="gslb")
                    nc.vector.tensor_copy(gslb[:], gsl[:])
                    dump(gslb[:], 5 * 128)
                    dump(memsl_n[:], 5 * 128)
                if s < NSEG - 1:
                    memstage = wrk.tile([128, NT * 128], bf16, tag="attn", bufs=2)
                    nc.vector.memset(memstage[:], 0.0)
                    nc.vector.tensor_copy(
                        memstage[:, bass.ds(toff, SL)], memsl_n[:])
                    kv_partials_and_ar2(memsl_n, memstage)

    nc.compile()
    return nc


def _prep_inputs(inputs):
    hs = np.asarray(inputs["hidden_states"], np.float32)
    Bsz = hs.shape[0]

    def bf(x):
        return np.ascontiguousarray(np.asarray(x, np.float32).astype(BF16))

    shared = {
        "wqr": bf(inputs["wq_r"]), "wgr": bf(inputs["wg_r"]),
        "wqw": bf(inputs["wq_w"]), "wor": bf(inputs["wo_r"]),
        "wkw": bf(inputs["wk_w"]), "wvw": bf(inputs["wv_w"]),
        "wow": bf(inputs["wo_w"]),
        "bgr": np.ascontiguousarray(
            np.asarray(inputs["bg_r"], np.float32)[:, None]),
        "mqt": bf(np.asarray(inputs["write_queries"], np.float32)[0].T),
        "m0t": bf(np.asarray(inputs["initial_memory"], np.float32)[0].T),
    }
    wgw = np.asarray(inputs["wg_w"], np.float32)
    wkr = np.asarray(inputs["wk_r"], np.float32)
    wvr = np.asarray(inputs["wv_r"], np.float32)
    bgw = np.asarray(inputs["bg_w"], np.float32)

    in_maps = []
    for c in range(8):
        b, t = c // 4, c % 4
        cols = np.concatenate(
            [np.arange(s * 1024 + t * TOK, s * 1024 + (t + 1) * TOK)
             for s in range(NSEG)])
        xt = np.ascontiguousarray(hs[b, cols, :].T)  # [D, 1024]
        im = dict(shared)
        im["xt32"] = xt
        im["xtb"] = np.ascontiguousarray(xt.astype(BF16))
        im["wgw_sl"] = np.ascontiguousarray(
            wgw[:, t * SL:(t + 1) * SL].astype(BF16))
        im["wkr_sl"] = np.ascontiguousarray(
            wkr[t * SL:(t + 1) * SL, :].astype(BF16))
        im["wvr_sl"] = np.ascontiguousarray(
            wvr[t * SL:(t + 1) * SL, :].astype(BF16))
        im["bgw_sl"] = np.ascontiguousarray(bgw[t * SL:(t + 1) * SL, None])
        in_maps.append(im)
    return in_maps


def _run(inputs, trace=False, debug=False):
    from concourse.bass_utils import run_bass_kernel_spmd
    key = ("nc", debug)
    if key not in _CACHE:
        _CACHE[key] = _build(debug=debug)
    in_maps = _prep_inputs(inputs)
    res = run_bass_kernel_spmd(_CACHE[key], in_maps, list(range(8)),
                               trace=trace)
    hs = np.asarray(inputs["hidden_states"])
    out = np.empty((hs.shape[0], NSEG * 1024, D), np.float32)
    for c in range(8):
        b, t = c // 4, c % 4
        htc = res.results[c]["ht"]  # [D, NSEG*TOK]
        for s in range(NSEG):
            out[b, s * 1024 + t * TOK:s * 1024 + (t + 1) * TOK, :] = \
                htc[:, s * TOK:(s + 1) * TOK].T
    return out, res


def kernel(**inputs):
    out, _ = _run(inputs, trace=False)
    return out



# revision 18
# speedup vs baseline: 214.7517x; 214.7517x over previous
"""RMT memory kernel for 8 Trainium2 NeuronCores — v2.

Core c = (batch b=c//4, token-quarter t=c%4); 256 tokens/segment/core.
All big gemms run fp8(e4m3)+DoubleRow with host-prescaled weights; the
descales are folded into activation scales / softmax reciprocals so they
cost nothing. Write-attention scores use the host-precomputed per-head
P = wk_w[:,h] @ (write_queries@wq_w)[:,h]^T, eliminating the k_w gemm.
Segment-0 memory update is a plain copy (reference semantics) and
segment 3 computes only read-attention + h (its memory update is dead).
q_r/gate projections (phase-1) are computed per segment into an SBUF
ring; chunks for segments 2,3 are emitted after the collective triggers
of segments 0,1 so the PE works through the AllReduce waits.
"""

import sys

sys.path.insert(0, "/opt/trn_rl_repo")

import numpy as np
import ml_dtypes

D = 2560
H = 8
HD = 320
MEM = 128
NSEG = 4
TOK = 256
NT = D // 128        # 20
SL = 640             # per-core D slice of the memory state
NSL = SL // 128      # 5
GROUPS = [[0, 1, 2, 3], [4, 5, 6, 7]]
SCALE = 1.0 / float(np.sqrt(HD))
BF16 = ml_dtypes.bfloat16
F8 = ml_dtypes.float8_e4m3

# fp8 pre-scales (host multiplies, kernel folds the inverse into cheap spots)
SW = 256.0   # all big weights
SX = 4.0     # hidden states
SC = 16.0    # ctx
SH = 8.0     # h
SP = 2048.0  # P (its entries are ~1e-3)
SA = 16.0    # normalized write-attention
SM = 16.0    # memory state as wgw/kv operand

_CACHE = {}


def _pieces(start, size):
    out, p, end = [], start, start + size
    while p < end:
        nxt = min(end, (p // 128 + 1) * 128)
        out.append((p, nxt - p))
        p = nxt
    return out


def _build():
    import concourse.bass as bass
    import concourse.bacc as bacc
    import concourse.mybir as mybir
    import concourse.tile as tile

    f32 = mybir.dt.float32
    bf16 = mybir.dt.bfloat16
    f8 = mybir.dt.float8e4
    AF = mybir.ActivationFunctionType
    DR = mybir.MatmulPerfMode.DoubleRow

    nc = bacc.Bacc("TRN2", target_bir_lowering=False, debug=False, num_devices=8)

    # ---------------- DRAM I/O ----------------
    x8d = nc.dram_tensor("x8", [D, NSEG * TOK], f8, kind="ExternalInput")
    xtbd = nc.dram_tensor("xtb", [D, NSEG * TOK], bf16, kind="ExternalInput")
    wqr8 = nc.dram_tensor("wqr8", [D, D], f8, kind="ExternalInput")
    wgr8 = nc.dram_tensor("wgr8", [D, D], f8, kind="ExternalInput")
    wor8 = nc.dram_tensor("wor8", [D, D], f8, kind="ExternalInput")
    wvw8 = nc.dram_tensor("wvw8", [D, D], f8, kind="ExternalInput")
    wow8 = nc.dram_tensor("wow8", [D, D], f8, kind="ExternalInput")
    wgw8 = nc.dram_tensor("wgw8", [128, NSL * 40 * 128], f8,
                          kind="ExternalInput")  # host slab-major per m-tile
    wkr8 = nc.dram_tensor("wkr8", [SL, D], f8, kind="ExternalInput")
    wvr8 = nc.dram_tensor("wvr8", [SL, D], f8, kind="ExternalInput")
    P8d = nc.dram_tensor("P8", [D, H * MEM], f8, kind="ExternalInput")
    kr0d = nc.dram_tensor("kr0", [128, NT * 128], bf16, kind="ExternalInput")
    vr0d = nc.dram_tensor("vr0", [MEM, D], bf16, kind="ExternalInput")
    bgrd = nc.dram_tensor("bgr", [128, NT], f32, kind="ExternalInput")
    bgwd = nc.dram_tensor("bgw", [128, NSL], f32, kind="ExternalInput")

    ht = nc.dram_tensor("ht", [D, NSEG * TOK], bf16, kind="ExternalOutput")

    def re3(ap, p=128):
        """[R, C] dram slice -> [p, R//p, C] slab-major view."""
        return ap.rearrange("(j p) m -> p j m", p=p)

    with tile.TileContext(nc) as tc:
        with (
            tc.tile_pool(name="dram", bufs=1, space="DRAM") as dram,
            tc.tile_pool(name="res", bufs=1) as res,
            tc.tile_pool(name="wrk", bufs=1) as wrk,
            tc.tile_pool(name="stream", bufs=1) as stream,
            tc.tile_pool(name="ps", bufs=1, space="PSUM") as psp,
        ):
            b1_in = dram.tile([129, NT * 128], bf16)
            b1_out = dram.tile([129, NT * 128], bf16)
            b2_in = dram.tile([384, NT * 128], bf16)
            b2_out = dram.tile([384, NT * 128], bf16)
            b2s_in = dram.tile([256, NT * 128], bf16)   # last AR2: no memstage
            b2s_out = dram.tile([256, NT * 128], bf16)

            def ps512():
                return psp.tile([128, 512], f32, tag="ps512", bufs=3, name="ps512")

            def ps256():
                return psp.tile([128, 256], f32, tag="ps256", bufs=5, name="ps256")

            # ---------------- residents ----------------
            krT = res.tile([128, NT * 128], bf16)   # k_r^T slab-major
            vr = res.tile([128, D], bf16)           # v_r [M, D]
            ones_c = res.tile([128, 1], bf16)       # read-denominator lhsT
            ones_w = res.tile([128, 1], bf16)       # write-denominator lhsT
            bgrt = res.tile([128, NT], f32)
            bgwt = res.tile([128, NSL], f32)
            nc.vector.memset(ones_c[:], 1.0 / SC)
            nc.vector.memset(ones_w[:], 1.0 / SA)
            nc.sync.dma_start(krT[:], kr0d[:, :])
            nc.sync.dma_start(vr[:], vr0d[:, :])
            nc.sync.dma_start(bgrt[:], bgrd[:, :])
            nc.sync.dma_start(bgwt[:], bgwd[:, :])

            pid = nc.vector.partition_id()
            toff = nc.snap((pid % 4) * SL, donate=False, min_val=0,
                           max_val=3 * SL)

            def wtile():
                return stream.tile([128, NT, 512], f8, tag="w8", bufs=2, name="wt")

            # ---------------- phase-1 chunks ----------------
            qr_slots = {}
            gt_slots = {}

            def phase1(s, which):
                x8t = stream.tile([128, NT, TOK], f8, tag="x8seg", bufs=1, name="x8t")
                nc.scalar.dma_start(
                    x8t[:], re3(x8d[:, s * TOK:(s + 1) * TOK]))
                w_dram = wqr8 if which == "q" else wgr8
                dst = wrk.tile([128, NT * TOK], bf16, name="p1dst",
                               tag="qrT" if which == "q" else "gtT", bufs=2)
                (qr_slots if which == "q" else gt_slots)[s] = dst
                for m0 in range(0, NT, 4):
                    wt = wtile()
                    nc.sync.dma_start(
                        wt[:], re3(w_dram[:, m0 * 128:(m0 + 4) * 128]))
                    for mi in range(4):
                        m = m0 + mi
                        ps = ps256()
                        for j in range(NT // 2):
                            nc.tensor.matmul(
                                ps, wt[:, 2 * j:2 * j + 2,
                                       mi * 128:(mi + 1) * 128],
                                x8t[:, 2 * j:2 * j + 2, :],
                                start=(j == 0), stop=(j == NT // 2 - 1),
                                perf_mode=DR)
                        if which == "q":
                            nc.scalar.activation(
                                dst[:, m * TOK:(m + 1) * TOK], ps, AF.Copy,
                                scale=1.0 / (SX * SW))
                        else:
                            nc.scalar.activation(
                                dst[:, m * TOK:(m + 1) * TOK], ps, AF.Sigmoid,
                                bias=bgrt[:, m:m + 1], scale=1.0 / (SX * SW))

            phase1(0, "q")
            phase1(0, "g")
            phase1(1, "q")
            phase1(1, "g")

            # ---------------- segment chain ----------------
            mem_slbf = None   # own 640-slice of current memory (bf16)
            memT8 = None      # full memory^T * SM (fp8), for wgw
            for s in range(NSEG):
                qrT = qr_slots.pop(s)
                gtT = gt_slots.pop(s)
                xtbt = stream.tile([128, NT, TOK], bf16, tag="xtbseg", bufs=1)
                nc.scalar.dma_start(
                    xtbt[:], re3(xtbd[:, s * TOK:(s + 1) * TOK]))

                # -- read attention scores + softmax pieces --
                E_sb = wrk.tile([128, H * TOK], bf16, tag="E")
                for h in range(H):
                    ps = ps256()
                    pcs = _pieces(h * HD, HD)
                    for i, (off, sz) in enumerate(pcs):
                        g, o = off // 128, off % 128
                        nc.tensor.matmul(
                            ps, krT[o:o + sz, g * 128:(g + 1) * 128],
                            qrT[o:o + sz, g * TOK:(g + 1) * TOK],
                            start=(i == 0), stop=(i == len(pcs) - 1))
                    nc.scalar.activation(E_sb[:, h * TOK:(h + 1) * TOK], ps,
                                         AF.Exp, scale=SCALE / (SM * SW))
                dpack = wrk.tile([64, H * TOK], f32, tag="dpack")
                dens = dpack[0:1, :]
                for hp in range(H // 2):
                    ps = ps512()
                    for hh in range(2):
                        h = hp * 2 + hh
                        nc.tensor.matmul(
                            ps[0:1, hh * TOK:(hh + 1) * TOK], ones_c[:],
                            E_sb[:, h * TOK:(h + 1) * TOK],
                            start=True, stop=True)
                    nc.vector.tensor_copy(
                        dens[:, hp * 512:(hp + 1) * 512], ps[0:1, :])
                recip = wrk.tile([1, H * TOK], f32, tag="recip")
                nc.vector.reciprocal(recip[:], dens[:])  # = SC / sum(E)
                # v_r is stored scaled by SM*SW; fold the descale in here
                nc.vector.tensor_scalar_mul(recip[:], recip[:],
                                            1.0 / (SM * SW))

                # -- ctx (normalized+scaled to fp8 at evacuation) --
                # rrep[h] (the per-head column-broadcast reciprocal) is
                # emitted just before head h's ctx matmuls so the psrep
                # ring (bufs=2) never makes the PE wait on DVE work that
                # is queued after it.
                ctx8 = wrk.tile([128, NT, TOK], f8, tag="ctx8")
                ctx_ps = {}
                rrep = {}
                for h in range(H):
                    rrep[h] = wrk.tile([128, TOK], f32, tag="rrep", bufs=2,
                                       name="rrep_h")
                    nc.gpsimd.partition_broadcast(
                        rrep[h][:], recip[:, h * TOK:(h + 1) * TOK])
                    for off, sz in _pieces(h * HD, HD):
                        g, o = off // 128, off % 128
                        if g not in ctx_ps:
                            ctx_ps[g] = (ps256(), [])
                        ps, evl = ctx_ps[g]
                        nc.tensor.matmul(
                            ps[o:o + sz, :], vr[:, off:off + sz],
                            E_sb[:, h * TOK:(h + 1) * TOK],
                            start=True, stop=True)
                        evl.append((h, o, sz))
                        if o + sz == 128 or (h == H - 1 and off + sz == D):
                            for (hh, oo, ss) in evl:
                                nc.vector.tensor_mul(
                                    ctx8[oo:oo + ss, g, :],
                                    ps[oo:oo + ss, :],
                                    rrep[hh][oo:oo + ss, :])
                            ctx_ps[g] = None

                # -- o = wo_r.T @ ctx ; h = seg + gate*o --
                h8 = wrk.tile([128, NT, TOK], f8, tag="h8")
                for m0 in range(0, NT, 4):
                    wt = wtile()
                    nc.sync.dma_start(
                        wt[:], re3(wor8[:, m0 * 128:(m0 + 4) * 128]))
                    for mi in range(4):
                        m = m0 + mi
                        ps = ps256()
                        for j in range(NT // 2):
                            nc.tensor.matmul(
                                ps, wt[:, 2 * j:2 * j + 2,
                                       mi * 128:(mi + 1) * 128],
                                ctx8[:, 2 * j:2 * j + 2, :],
                                start=(j == 0), stop=(j == NT // 2 - 1),
                                perf_mode=DR)
                        obf = wrk.tile([128, TOK], bf16, tag="obf", bufs=2)
                        nc.scalar.activation(obf[:], ps, AF.Copy,
                                             scale=1.0 / (SW * SC))
                        tmp32 = wrk.tile([128, TOK], f32, tag="tmp32", bufs=2)
                        nc.vector.tensor_mul(
                            tmp32[:], obf[:], gtT[:, m * TOK:(m + 1) * TOK])
                        hbf = wrk.tile([128, TOK], bf16, tag="hbf", bufs=2)
                        nc.vector.tensor_add(hbf[:], tmp32[:], xtbt[:, m, :])
                        nc.scalar.dma_start(
                            ht[m * 128:(m + 1) * 128,
                               s * TOK:(s + 1) * TOK], hbf[:])
                        nc.vector.tensor_scalar_mul(h8[:, m, :], hbf[:], SH)

                if s == NSEG - 1:
                    break

                # -- write-attention scores via P:  ewT = exp(h^T @ P) --
                ewT = wrk.tile([128, 2 * H * 128], bf16, tag="ewT")

                def ewcol(h, tt):
                    return tt * 1024 + (h // 4) * 512 + (h % 4) * 128

                for hh in range(2):
                    pt = wtile()
                    nc.scalar.dma_start(
                        pt[:], re3(P8d[:, hh * 512:(hh + 1) * 512]))
                    for tt in range(2):
                        ps = ps512()
                        for j in range(NT // 2):
                            nc.tensor.matmul(
                                ps, h8[:, 2 * j:2 * j + 2,
                                       tt * 128:(tt + 1) * 128],
                                pt[:, 2 * j:2 * j + 2, :],
                                start=(j == 0), stop=(j == NT // 2 - 1),
                                perf_mode=DR)
                        nc.scalar.activation(
                            ewT[:, tt * 1024 + hh * 512:
                                tt * 1024 + (hh + 1) * 512],
                            ps, AF.Exp, scale=SCALE / (SH * SP))

                # -- v_w gemm --
                vw = wrk.tile([128, 2 * D], bf16, tag="vw")
                for ch in range(5):
                    wt = wtile()
                    nc.sync.dma_start(
                        wt[:], re3(wvw8[:, ch * 512:(ch + 1) * 512]))
                    for tt in range(2):
                        ps = ps512()
                        for j in range(NT // 2):
                            nc.tensor.matmul(
                                ps, h8[:, 2 * j:2 * j + 2,
                                       tt * 128:(tt + 1) * 128],
                                wt[:, 2 * j:2 * j + 2, :],
                                start=(j == 0), stop=(j == NT // 2 - 1),
                                perf_mode=DR)
                        nc.scalar.activation(
                            vw[:, tt * D + ch * 512:tt * D + (ch + 1) * 512],
                            ps, AF.Copy, scale=1.0 / (SW * SH))

                # -- write numerators / denominators --
                attnT = wrk.tile([128, NT * 128], bf16, tag="abuf", bufs=2)
                att_ps = {}
                for h in range(H):
                    for off, sz in _pieces(h * HD, HD):
                        g, o = off // 128, off % 128
                        if g not in att_ps:
                            att_ps[g] = (ps256(), [])
                        ps, evl = att_ps[g]
                        for tt in range(2):
                            nc.tensor.matmul(
                                ps[o:o + sz, 0:128],
                                vw[:, tt * D + off:tt * D + off + sz],
                                ewT[:, ewcol(h, tt):ewcol(h, tt) + 128],
                                start=(tt == 0), stop=(tt == 1))
                        evl.append((h, o, sz))
                        if o + sz == 128 or (h == H - 1 and off + sz == D):
                            for (hh, oo, ss) in evl:
                                nc.scalar.copy(
                                    attnT[oo:oo + ss, g * 128:(g + 1) * 128],
                                    ps[oo:oo + ss, 0:128])
                            att_ps[g] = None
                densw = dpack[32:33, 0:H * 128]
                for hp in range(H // 4):
                    ps = ps512()
                    for hh in range(4):
                        h = hp * 4 + hh
                        for tt in range(2):
                            nc.tensor.matmul(
                                ps[0:1, hh * 128:(hh + 1) * 128], ones_w[:],
                                ewT[:, ewcol(h, tt):ewcol(h, tt) + 128],
                                start=(tt == 0), stop=(tt == 1))
                    nc.vector.tensor_copy(
                        densw[:, hp * 512:(hp + 1) * 512], ps[0:1, :])
                dbpack = wrk.tile([64, H * 128], bf16, tag="dbpack")
                nc.vector.tensor_copy(dbpack[0:1, :], densw[:])

                # -- AR#1 --
                nc.gpsimd.dma_start(b1_in[0:128, :], attnT[:])
                nc.gpsimd.dma_start(b1_in[128:129, 0:H * 128], dbpack[0:1, :])
                nc.gpsimd.collective_compute(
                    "AllReduce", mybir.AluOpType.add,
                    ins=[b1_in.opt()], outs=[b1_out.opt()],
                    replica_groups=GROUPS)

                # prefetch the k_r/v_r projection weights for the tail
                # (3 column-chunks of 1024/1024/512 per weight)
                def kvchunk(wsrc, ci):
                    c0, c1 = ci * 1024, min((ci + 1) * 1024, D)
                    t = stream.tile([128, NSL, 1024], f8, tag="wkv", bufs=3,
                                    name="kvw")
                    nc.sync.dma_start(t[:, :, 0:c1 - c0], re3(wsrc[:, c0:c1]))
                    return t
                wkt = [kvchunk(wkr8, ci) for ci in range(3)]

                # filler: next-next segment q_r projection under the AR wait
                if s + 2 < NSEG:
                    phase1(s + 2, "q")

                attnS = wrk.tile([128, NT * 128], bf16, tag="abuf", bufs=2)
                nc.gpsimd.dma_start(attnS[:], b1_out[0:128, :])
                nc.gpsimd.dma_start(dbpack[32:33, :], b1_out[128:129, 0:H * 128])
                denswS = wrk.tile([1, H * 128], f32, tag="denswS")
                nc.vector.tensor_copy(denswS[:], dbpack[32:33, :])
                recw = wrk.tile([1, H * 128], f32, tag="recw")
                nc.vector.reciprocal(recw[:], denswS[:])  # = SA / densum
                attnN8 = wrk.tile([128, NT, 128], f8, tag="attnN8")
                rwrep = {}
                for h in range(H):
                    rwrep[h] = wrk.tile([128, 128], f32, tag="rwrep", bufs=2,
                                        name="rwrep_h")
                    nc.gpsimd.partition_broadcast(
                        rwrep[h][:], recw[:, h * 128:(h + 1) * 128])
                    for off, sz in _pieces(h * HD, HD):
                        g, o = off // 128, off % 128
                        nc.vector.tensor_mul(
                            attnN8[o:o + sz, g, :],
                            attnS[o:o + sz, g * 128:(g + 1) * 128],
                            rwrep[h][o:o + sz, :])

                # -- new_mem^T = wo_w.T @ attnN (replicated) --
                new_sb = wrk.tile([128, NT * 128], bf16, tag="new")
                for m0 in range(0, NT, 4):
                    wt = wtile()
                    nc.sync.dma_start(
                        wt[:], re3(wow8[:, m0 * 128:(m0 + 4) * 128]))
                    for mi in range(4):
                        m = m0 + mi
                        ps = ps256()
                        for j in range(NT // 2):
                            nc.tensor.matmul(
                                ps[:, 0:128],
                                wt[:, 2 * j:2 * j + 2,
                                   mi * 128:(mi + 1) * 128],
                                attnN8[:, 2 * j:2 * j + 2, :],
                                start=(j == 0), stop=(j == NT // 2 - 1),
                                perf_mode=DR)
                        nc.scalar.activation(
                            new_sb[:, m * 128:(m + 1) * 128], ps[:, 0:128],
                            AF.Copy, scale=1.0 / (SW * SA))
                newT8 = wrk.tile([128, NT, 128], f8, tag="newT8", bufs=2)
                nc.vector.tensor_scalar_mul(
                    newT8[:].rearrange("p j m -> p (j m)"), new_sb[:], SM)

                # -- memory update (own 640-slice) --
                new_sl = new_sb[:, bass.ds(toff, SL)]
                if s == 0:
                    msl = wrk.tile([128, SL], bf16, tag="mslbf", bufs=2)
                    nc.vector.tensor_copy(msl[:], new_sl)
                    mem_slbf = msl
                    memT8 = newT8
                else:
                    # wgw gate gemm: one host-prearranged [5120, 128]
                    # m-tile stream at a time; slabs 0..19 contract the
                    # memory half, 20..39 the new_mem half.
                    gsl = wrk.tile([128, SL], f32, tag="gsl")
                    for m in range(NSL):
                        wt = stream.tile([128, 40, 128], f8, tag="w8",
                                         bufs=2, name="wgwt")
                        nc.sync.dma_start(
                            wt[:].rearrange("p j m -> p (j m)"),
                            wgw8[:, m * 5120:(m + 1) * 5120])
                        ps = ps256()
                        for half, rhs8 in ((0, memT8), (1, newT8)):
                            for j in range(NT // 2):
                                nc.tensor.matmul(
                                    ps[:, 0:128],
                                    wt[:, half * NT + 2 * j:
                                       half * NT + 2 * j + 2, :],
                                    rhs8[:, 2 * j:2 * j + 2, :],
                                    start=(half == 0 and j == 0),
                                    stop=(half == 1 and j == NT // 2 - 1),
                                    perf_mode=DR)
                        nc.scalar.activation(
                            gsl[:, m * 128:(m + 1) * 128], ps[:, 0:128],
                            AF.Sigmoid, bias=bgwt[:, m:m + 1],
                            scale=1.0 / (SW * SM))
                    dlt = wrk.tile([128, SL], f32, tag="dlt")
                    nc.vector.tensor_sub(dlt[:], new_sl, mem_slbf[:])
                    nc.vector.tensor_mul(dlt[:], gsl[:], dlt[:])
                    msl = wrk.tile([128, SL], bf16, tag="mslbf", bufs=2)
                    nc.vector.tensor_add(msl[:], mem_slbf[:], dlt[:])
                    mem_slbf = msl

                # -- k_r/v_r partials from the updated slice + AR#2 --
                memsl8 = wrk.tile([128, NSL, 128], f8, tag="memsl8")
                nc.vector.tensor_scalar_mul(
                    memsl8[:].rearrange("p j m -> p (j m)"), mem_slbf[:], SM)
                pk = wrk.tile([128, NT * 128], bf16, tag="abuf", bufs=2)
                for g in range(NT):
                    t = wkt[g // 8]
                    gc = (g % 8) * 128
                    ps = ps256()
                    for j in range(2):
                        nc.tensor.matmul(
                            ps[:, 0:128],
                            t[:, 2 * j:2 * j + 2, gc:gc + 128],
                            memsl8[:, 2 * j:2 * j + 2, :],
                            start=(j == 0), stop=False, perf_mode=DR)
                    nc.tensor.matmul(
                        ps[:, 0:128], t[:, 4, gc:gc + 128],
                        memsl8[:, 4, :], start=False, stop=True)
                    nc.scalar.copy(pk[:, g * 128:(g + 1) * 128], ps[:, 0:128])
                pv = wrk.tile([128, NT * 128], bf16, tag="abuf", bufs=2)
                wvt = [kvchunk(wvr8, ci) for ci in range(3)]
                for ch in range(5):
                    t = wvt[ch // 2]
                    cc = (ch % 2) * 512
                    ps = ps512()
                    for j in range(2):
                        nc.tensor.matmul(
                            ps, memsl8[:, 2 * j:2 * j + 2, :],
                            t[:, 2 * j:2 * j + 2, cc:cc + 512],
                            start=(j == 0), stop=False, perf_mode=DR)
                    nc.tensor.matmul(
                        ps, memsl8[:, 4, :],
                        t[:, 4, cc:cc + 512],
                        start=False, stop=True)
                    nc.scalar.copy(pv[:, ch * 512:(ch + 1) * 512], ps)
                # full mem_{s+1} is only needed (by segment s+1's wgw gemm)
                # when it isn't already replicated: at s=0 mem_1 = new_mem
                # (replicated), at s=2 mem_3 is never used beyond k_r/v_r.
                need_mem_bcast = s == 1
                bi, bo = (b2_in, b2_out) if need_mem_bcast else (b2s_in,
                                                                 b2s_out)
                if need_mem_bcast:
                    stagem = wrk.tile([128, NT * 128], bf16, tag="stage")
                    nc.vector.memset(stagem[:], 0.0)
                    nc.vector.tensor_copy(stagem[:, bass.ds(toff, SL)],
                                          mem_slbf[:])
                    nc.gpsimd.dma_start(bi[256:384, :], stagem[:])
                nc.gpsimd.dma_start(bi[0:128, :], pk[:])
                nc.gpsimd.dma_start(bi[128:256, :], pv[:])
                nc.gpsimd.collective_compute(
                    "AllReduce", mybir.AluOpType.add,
                    ins=[bi.opt()], outs=[bo.opt()],
                    replica_groups=GROUPS)

                # filler: next-next segment gate projection under the AR wait
                if s + 2 < NSEG:
                    phase1(s + 2, "g")

                nc.gpsimd.dma_start(krT[:], bo[0:128, :])
                nc.gpsimd.dma_start(vr[:], bo[128:256, :])
                if need_mem_bcast and s >= 1:
                    memTbf = wrk.tile([128, NT * 128], bf16, tag="stage")
                    nc.gpsimd.dma_start(memTbf[:], bo[256:384, :])
                    memT8 = wrk.tile([128, NT, 128], f8, tag="newT8", bufs=2)
                    nc.vector.tensor_scalar_mul(
                        memT8[:].rearrange("p j m -> p (j m)"), memTbf[:], SM)

    nc.compile()
    return nc


def _prep_inputs(inputs):
    hs = np.asarray(inputs["hidden_states"], np.float32)

    def f8w(x):
        return np.ascontiguousarray(
            (np.asarray(x, np.float32) * SW).astype(F8))

    wq_w = np.asarray(inputs["wq_w"], np.float32)
    wk_w = np.asarray(inputs["wk_w"], np.float32)
    wk_r = np.asarray(inputs["wk_r"], np.float32)
    wv_r = np.asarray(inputs["wv_r"], np.float32)
    wg_w = np.asarray(inputs["wg_w"], np.float32)
    im = np.asarray(inputs["initial_memory"], np.float32)[0]     # [M, D]
    mq = np.asarray(inputs["write_queries"], np.float32)[0]      # [M, D]

    qw = mq @ wq_w                                               # [M, D]
    P = np.empty((D, H * MEM), np.float32)
    for h in range(H):
        hsl = slice(h * HD, (h + 1) * HD)
        P[:, h * MEM:(h + 1) * MEM] = wk_w[:, hsl] @ qw[:, hsl].T
    P8 = np.ascontiguousarray((P * SP).astype(F8))

    kr0 = (im @ wk_r) * (SM * SW)   # scaled like the AR'd partials
    # slab-major k_r^T: krT[p, j*128+m] = kr0[m, j*128+p]
    kr0_sm = np.ascontiguousarray(
        kr0.T.reshape(NT, 128, MEM).transpose(1, 0, 2).reshape(128, NT * MEM)
        .astype(BF16))
    vr0 = np.ascontiguousarray(((im @ wv_r) * (SM * SW)).astype(BF16))

    bgr = np.asarray(inputs["bg_r"], np.float32)                 # [D]
    bgr_t = np.ascontiguousarray(bgr.reshape(NT, 128).T)         # [128, NT]
    bgw = np.asarray(inputs["bg_w"], np.float32)

    shared = {
        "wqr8": f8w(inputs["wq_r"]), "wgr8": f8w(inputs["wg_r"]),
        "wor8": f8w(inputs["wo_r"]), "wvw8": f8w(inputs["wv_w"]),
        "wow8": f8w(inputs["wo_w"]),
        "P8": P8, "kr0": kr0_sm, "vr0": vr0, "bgr": bgr_t,
    }

    in_maps = []
    for c in range(8):
        b, t = c // 4, c % 4
        cols = np.concatenate(
            [np.arange(s * 1024 + t * TOK, s * 1024 + (t + 1) * TOK)
             for s in range(NSEG)])
        xt = np.ascontiguousarray(hs[b, cols, :].T)              # [D, 1024]
        imap = dict(shared)
        imap["x8"] = np.ascontiguousarray((xt * SX).astype(F8))
        imap["xtb"] = np.ascontiguousarray(xt.astype(BF16))
        wgsl = (wg_w[:, t * SL:(t + 1) * SL] * SW).astype(F8)  # [2D, SL]
        # slab-major per m-tile: [128, (m, j, c)] with w[j*128+p, m*128+c]
        imap["wgw8"] = np.ascontiguousarray(
            wgsl.reshape(40, 128, NSL, 128).transpose(1, 2, 0, 3)
            .reshape(128, NSL * 40 * 128))
        imap["wkr8"] = f8w(wk_r[t * SL:(t + 1) * SL, :])
        imap["wvr8"] = f8w(wv_r[t * SL:(t + 1) * SL, :])
        imap["bgw"] = np.ascontiguousarray(
            bgw[t * SL:(t + 1) * SL].reshape(NSL, 128).T)        # [128, NSL]
        in_maps.append(imap)
    return in_maps


def _run(inputs, trace=False):
    from concourse.bass_utils import run_bass_kernel_spmd
    if "nc" not in _CACHE:
        _CACHE["nc"] = _build()
    in_maps = _prep_inputs(inputs)
    res = run_bass_kernel_spmd(_CACHE["nc"], in_maps, list(range(8)),
                               trace=trace)
    hs = np.asarray(inputs["hidden_states"])
    out = np.empty((hs.shape[0], NSEG * 1024, D), np.float32)
    for c in range(8):
        b, t = c // 4, c % 4
        htc = np.asarray(res.results[c]["ht"], np.float32)       # [D, 1024]
        for s in range(NSEG):
            out[b, s * 1024 + t * TOK:s * 1024 + (t + 1) * TOK, :] = \
                htc[:, s * TOK:(s + 1) * TOK].T
    return out, res


def kernel(**inputs):
    out, _ = _run(inputs, trace=False)
    return out


# revision 20
# speedup vs baseline: 228.3498x; 1.0633x over previous
"""RMT memory kernel for 8 Trainium2 NeuronCores — v2.

Core c = (batch b=c//4, token-quarter t=c%4); 256 tokens/segment/core.
All big gemms run fp8(e4m3)+DoubleRow with host-prescaled weights; the
descales are folded into activation scales / softmax reciprocals so they
cost nothing. Write-attention scores use the host-precomputed per-head
P = wk_w[:,h] @ (write_queries@wq_w)[:,h]^T, eliminating the k_w gemm.
Segment-0 memory update is a plain copy (reference semantics) and
segment 3 computes only read-attention + h (its memory update is dead).
q_r/gate projections (phase-1) are computed per segment into an SBUF
ring; chunks for segments 2,3 are emitted after the collective triggers
of segments 0,1 so the PE works through the AllReduce waits.
"""

import sys

sys.path.insert(0, "/opt/trn_rl_repo")

import numpy as np
import ml_dtypes

D = 2560
H = 8
HD = 320
MEM = 128
NSEG = 4
TOK = 256
NT = D // 128        # 20
SL = 640             # per-core D slice of the memory state
NSL = SL // 128      # 5
GROUPS = [[0, 1, 2, 3], [4, 5, 6, 7]]
SCALE = 1.0 / float(np.sqrt(HD))
BF16 = ml_dtypes.bfloat16
F8 = ml_dtypes.float8_e4m3

# fp8 pre-scales (host multiplies, kernel folds the inverse into cheap spots)
SW = 256.0   # all big weights
SX = 4.0     # hidden states
SC = 16.0    # ctx
SH = 8.0     # h
SP = 2048.0  # P (its entries are ~1e-3)
SA = 16.0    # normalized write-attention
SM = 16.0    # memory state as wgw/kv operand

_CACHE = {}


def _pieces(start, size):
    out, p, end = [], start, start + size
    while p < end:
        nxt = min(end, (p // 128 + 1) * 128)
        out.append((p, nxt - p))
        p = nxt
    return out


def _build():
    import concourse.bass as bass
    import concourse.bacc as bacc
    import concourse.mybir as mybir
    import concourse.tile as tile

    f32 = mybir.dt.float32
    bf16 = mybir.dt.bfloat16
    f8 = mybir.dt.float8e4
    AF = mybir.ActivationFunctionType
    DR = mybir.MatmulPerfMode.DoubleRow

    nc = bacc.Bacc("TRN2", target_bir_lowering=False, debug=False, num_devices=8)

    # ---------------- DRAM I/O ----------------
    x8d = nc.dram_tensor("x8", [D, NSEG * TOK], f8, kind="ExternalInput")
    xtbd = nc.dram_tensor("xtb", [D, NSEG * TOK], bf16, kind="ExternalInput")
    wqr8 = nc.dram_tensor("wqr8", [D, D], f8, kind="ExternalInput")
    wgr8 = nc.dram_tensor("wgr8", [D, D], f8, kind="ExternalInput")
    wor8 = nc.dram_tensor("wor8", [D, D], f8, kind="ExternalInput")
    wvw8 = nc.dram_tensor("wvw8", [D, D], f8, kind="ExternalInput")
    wow8 = nc.dram_tensor("wow8", [D, D], f8, kind="ExternalInput")
    wgw8 = nc.dram_tensor("wgw8", [128, NSL * 40 * 128], f8,
                          kind="ExternalInput")  # host slab-major per m-tile
    wkr8 = nc.dram_tensor("wkr8", [SL, D], f8, kind="ExternalInput")
    wvr8 = nc.dram_tensor("wvr8", [SL, D], f8, kind="ExternalInput")
    P8d = nc.dram_tensor("P8", [D, H * MEM], f8, kind="ExternalInput")
    kr0d = nc.dram_tensor("kr0", [128, NT * 128], bf16, kind="ExternalInput")
    vr0d = nc.dram_tensor("vr0", [MEM, D], bf16, kind="ExternalInput")
    bgrd = nc.dram_tensor("bgr", [128, NT], f32, kind="ExternalInput")
    bgwd = nc.dram_tensor("bgw", [128, NSL], f32, kind="ExternalInput")

    ht = nc.dram_tensor("ht", [D, NSEG * TOK], bf16, kind="ExternalOutput")

    def re3(ap, p=128):
        """[R, C] dram slice -> [p, R//p, C] slab-major view."""
        return ap.rearrange("(j p) m -> p j m", p=p)

    with tile.TileContext(nc) as tc:
        with (
            tc.tile_pool(name="dram", bufs=1, space="DRAM") as dram,
            tc.tile_pool(name="res", bufs=1) as res,
            tc.tile_pool(name="wrk", bufs=1) as wrk,
            tc.tile_pool(name="stream", bufs=1) as stream,
            tc.tile_pool(name="ps", bufs=1, space="PSUM") as psp,
        ):
            b1_in = dram.tile([129, NT * 128], bf16)
            b1_out = dram.tile([129, NT * 128], bf16)
            b2_in = dram.tile([384, NT * 128], bf16)
            b2_out = dram.tile([384, NT * 128], bf16)
            b2s_in = dram.tile([256, NT * 128], bf16)   # last AR2: no memstage
            b2s_out = dram.tile([256, NT * 128], bf16)

            def ps512():
                return psp.tile([128, 512], f32, tag="ps512", bufs=3, name="ps512")

            def ps256():
                return psp.tile([128, 256], f32, tag="ps256", bufs=5, name="ps256")

            # ---------------- residents ----------------
            krT = res.tile([128, NT * 128], bf16)   # k_r^T slab-major
            vr = res.tile([128, D], bf16)           # v_r [M, D]
            ones_c = res.tile([128, 1], bf16)       # read-denominator lhsT
            ones_w = res.tile([128, 1], bf16)       # write-denominator lhsT
            bgrt = res.tile([128, NT], f32)
            bgwt = res.tile([128, NSL], f32)
            nc.vector.memset(ones_c[:], 1.0 / SC)
            nc.vector.memset(ones_w[:], 1.0 / SA)
            nc.sync.dma_start(krT[:], kr0d[:, :])
            nc.sync.dma_start(vr[:], vr0d[:, :])
            nc.sync.dma_start(bgrt[:], bgrd[:, :])
            nc.sync.dma_start(bgwt[:], bgwd[:, :])

            pid = nc.vector.partition_id()
            toff = nc.snap((pid % 4) * SL, donate=False, min_val=0,
                           max_val=3 * SL)

            def wtile():
                return stream.tile([128, NT, 512], f8, tag="w8", bufs=3, name="wt")

            # ---------------- phase-1 chunks ----------------
            qr_slots = {}
            gt_slots = {}

            def phase1(s, which):
                x8t = stream.tile([128, NT, TOK], f8, tag="x8seg", bufs=1, name="x8t")
                nc.scalar.dma_start(
                    x8t[:], re3(x8d[:, s * TOK:(s + 1) * TOK]))
                w_dram = wqr8 if which == "q" else wgr8
                dst = wrk.tile([128, NT * TOK], bf16, name="p1dst",
                               tag="qrT" if which == "q" else "gtT", bufs=2)
                (qr_slots if which == "q" else gt_slots)[s] = dst
                for m0 in range(0, NT, 4):
                    wt = wtile()
                    nc.sync.dma_start(
                        wt[:], re3(w_dram[:, m0 * 128:(m0 + 4) * 128]))
                    for mi in range(4):
                        m = m0 + mi
                        ps = ps256()
                        for j in range(NT // 2):
                            nc.tensor.matmul(
                                ps, wt[:, 2 * j:2 * j + 2,
                                       mi * 128:(mi + 1) * 128],
                                x8t[:, 2 * j:2 * j + 2, :],
                                start=(j == 0), stop=(j == NT // 2 - 1),
                                perf_mode=DR)
                        if which == "q":
                            nc.scalar.activation(
                                dst[:, m * TOK:(m + 1) * TOK], ps, AF.Copy,
                                scale=1.0 / (SX * SW))
                        else:
                            nc.scalar.activation(
                                dst[:, m * TOK:(m + 1) * TOK], ps, AF.Sigmoid,
                                bias=bgrt[:, m:m + 1], scale=1.0 / (SX * SW))

            phase1(0, "q")
            phase1(0, "g")

            # ---------------- segment chain ----------------
            mem_slbf = None   # own 640-slice of current memory (bf16)
            memT8 = None      # full memory^T * SM (fp8), for wgw
            for s in range(NSEG):
                qrT = qr_slots.pop(s)
                gtT = gt_slots.pop(s)
                xtbt = stream.tile([128, NT, TOK], bf16, tag="xtbseg", bufs=1)
                nc.scalar.dma_start(
                    xtbt[:], re3(xtbd[:, s * TOK:(s + 1) * TOK]))

                # -- read attention scores + softmax pieces --
                E_sb = wrk.tile([128, H * TOK], bf16, tag="E")
                for h in range(H):
                    ps = ps256()
                    pcs = _pieces(h * HD, HD)
                    for i, (off, sz) in enumerate(pcs):
                        g, o = off // 128, off % 128
                        nc.tensor.matmul(
                            ps, krT[o:o + sz, g * 128:(g + 1) * 128],
                            qrT[o:o + sz, g * TOK:(g + 1) * TOK],
                            start=(i == 0), stop=(i == len(pcs) - 1))
                    nc.scalar.activation(E_sb[:, h * TOK:(h + 1) * TOK], ps,
                                         AF.Exp, scale=SCALE / (SM * SW))
                dpack = wrk.tile([64, H * TOK], f32, tag="dpack")
                dens = dpack[0:1, :]
                for hp in range(H // 2):
                    ps = ps512()
                    for hh in range(2):
                        h = hp * 2 + hh
                        nc.tensor.matmul(
                            ps[0:1, hh * TOK:(hh + 1) * TOK], ones_c[:],
                            E_sb[:, h * TOK:(h + 1) * TOK],
                            start=True, stop=True)
                    nc.vector.tensor_copy(
                        dens[:, hp * 512:(hp + 1) * 512], ps[0:1, :])
                recip = wrk.tile([1, H * TOK], f32, tag="recip")
                nc.vector.reciprocal(recip[:], dens[:])  # = SC / sum(E)
                # v_r is stored scaled by SM*SW; fold the descale in here
                nc.vector.tensor_scalar_mul(recip[:], recip[:],
                                            1.0 / (SM * SW))

                # -- ctx (normalized+scaled to fp8 at evacuation) --
                # rrep[h] (the per-head column-broadcast reciprocal) is
                # emitted just before head h's ctx matmuls so the psrep
                # ring (bufs=2) never makes the PE wait on DVE work that
                # is queued after it.
                ctx8 = wrk.tile([128, NT, TOK], f8, tag="ctx8")
                ctx_ps = {}
                rrep = {}
                for h in range(H):
                    rrep[h] = wrk.tile([128, TOK], f32, tag="rrep", bufs=2,
                                       name="rrep_h")
                    nc.gpsimd.partition_broadcast(
                        rrep[h][:], recip[:, h * TOK:(h + 1) * TOK])
                    for off, sz in _pieces(h * HD, HD):
                        g, o = off // 128, off % 128
                        if g not in ctx_ps:
                            ctx_ps[g] = (ps256(), [])
                        ps, evl = ctx_ps[g]
                        nc.tensor.matmul(
                            ps[o:o + sz, :], vr[:, off:off + sz],
                            E_sb[:, h * TOK:(h + 1) * TOK],
                            start=True, stop=True)
                        evl.append((h, o, sz))
                        if o + sz == 128 or (h == H - 1 and off + sz == D):
                            for (hh, oo, ss) in evl:
                                nc.vector.tensor_mul(
                                    ctx8[oo:oo + ss, g, :],
                                    ps[oo:oo + ss, :],
                                    rrep[hh][oo:oo + ss, :])
                            ctx_ps[g] = None

                # -- o = wo_r.T @ ctx ; h = seg + gate*o --
                h8 = wrk.tile([128, NT, TOK], f8, tag="h8")
                for m0 in range(0, NT, 4):
                    wt = wtile()
                    nc.sync.dma_start(
                        wt[:], re3(wor8[:, m0 * 128:(m0 + 4) * 128]))
                    for mi in range(4):
                        m = m0 + mi
                        ps = ps256()
                        for j in range(NT // 2):
                            nc.tensor.matmul(
                                ps, wt[:, 2 * j:2 * j + 2,
                                       mi * 128:(mi + 1) * 128],
                                ctx8[:, 2 * j:2 * j + 2, :],
                                start=(j == 0), stop=(j == NT // 2 - 1),
                                perf_mode=DR)
                        obf = wrk.tile([128, TOK], bf16, tag="obf", bufs=2)
                        nc.scalar.activation(obf[:], ps, AF.Copy,
                                             scale=1.0 / (SW * SC))
                        tmp32 = wrk.tile([128, TOK], f32, tag="tmp32", bufs=2)
                        nc.vector.tensor_mul(
                            tmp32[:], obf[:], gtT[:, m * TOK:(m + 1) * TOK])
                        hbf = wrk.tile([128, TOK], bf16, tag="hbf", bufs=2)
                        nc.vector.tensor_add(hbf[:], tmp32[:], xtbt[:, m, :])
                        nc.scalar.dma_start(
                            ht[m * 128:(m + 1) * 128,
                               s * TOK:(s + 1) * TOK], hbf[:])
                        nc.vector.tensor_scalar_mul(h8[:, m, :], hbf[:], SH)

                if s == NSEG - 1:
                    break

                # -- write-attention scores via P:  ewT = exp(h^T @ P) --
                ewT = wrk.tile([128, 2 * H * 128], bf16, tag="ewT")

                def ewcol(h, tt):
                    return tt * 1024 + (h // 4) * 512 + (h % 4) * 128

                for hh in range(2):
                    pt = wtile()
                    nc.scalar.dma_start(
                        pt[:], re3(P8d[:, hh * 512:(hh + 1) * 512]))
                    for tt in range(2):
                        ps = ps512()
                        for j in range(NT // 2):
                            nc.tensor.matmul(
                                ps, h8[:, 2 * j:2 * j + 2,
                                       tt * 128:(tt + 1) * 128],
                                pt[:, 2 * j:2 * j + 2, :],
                                start=(j == 0), stop=(j == NT // 2 - 1),
                                perf_mode=DR)
                        nc.scalar.activation(
                            ewT[:, tt * 1024 + hh * 512:
                                tt * 1024 + (hh + 1) * 512],
                            ps, AF.Exp, scale=SCALE / (SH * SP))

                # -- v_w gemm --
                vw = wrk.tile([128, 2 * D], bf16, tag="vw")
                for ch in range(5):
                    wt = wtile()
                    nc.scalar.dma_start(
                        wt[:], re3(wvw8[:, ch * 512:(ch + 1) * 512]))
                    for tt in range(2):
                        ps = ps512()
                        for j in range(NT // 2):
                            nc.tensor.matmul(
                                ps, h8[:, 2 * j:2 * j + 2,
                                       tt * 128:(tt + 1) * 128],
                                wt[:, 2 * j:2 * j + 2, :],
                                start=(j == 0), stop=(j == NT // 2 - 1),
                                perf_mode=DR)
                        nc.scalar.activation(
                            vw[:, tt * D + ch * 512:tt * D + (ch + 1) * 512],
                            ps, AF.Copy, scale=1.0 / (SW * SH))

                # -- write numerators / denominators --
                attnT = wrk.tile([128, NT * 128], bf16, tag="abuf", bufs=2)
                att_ps = {}
                for h in range(H):
                    for off, sz in _pieces(h * HD, HD):
                        g, o = off // 128, off % 128
                        if g not in att_ps:
                            att_ps[g] = (ps256(), [])
                        ps, evl = att_ps[g]
                        for tt in range(2):
                            nc.tensor.matmul(
                                ps[o:o + sz, 0:128],
                                vw[:, tt * D + off:tt * D + off + sz],
                                ewT[:, ewcol(h, tt):ewcol(h, tt) + 128],
                                start=(tt == 0), stop=(tt == 1))
                        evl.append((h, o, sz))
                        if o + sz == 128 or (h == H - 1 and off + sz == D):
                            for (hh, oo, ss) in evl:
                                nc.scalar.copy(
                                    attnT[oo:oo + ss, g * 128:(g + 1) * 128],
                                    ps[oo:oo + ss, 0:128])
                            att_ps[g] = None
                densw = dpack[32:33, 0:H * 128]
                for hp in range(H // 4):
                    ps = ps512()
                    for hh in range(4):
                        h = hp * 4 + hh
                        for tt in range(2):
                            nc.tensor.matmul(
                                ps[0:1, hh * 128:(hh + 1) * 128], ones_w[:],
                                ewT[:, ewcol(h, tt):ewcol(h, tt) + 128],
                                start=(tt == 0), stop=(tt == 1))
                    nc.vector.tensor_copy(
                        densw[:, hp * 512:(hp + 1) * 512], ps[0:1, :])
                dbpack = wrk.tile([64, H * 128], bf16, tag="dbpack")
                nc.vector.tensor_copy(dbpack[0:1, :], densw[:])

                # -- AR#1 --
                nc.gpsimd.dma_start(b1_in[0:128, :], attnT[:])
                nc.gpsimd.dma_start(b1_in[128:129, 0:H * 128], dbpack[0:1, :])
                nc.gpsimd.collective_compute(
                    "AllReduce", mybir.AluOpType.add,
                    ins=[b1_in.opt()], outs=[b1_out.opt()],
                    replica_groups=GROUPS)

                # prefetch the k_r/v_r projection weights for the tail
                # (3 column-chunks of 1024/1024/512 per weight)
                def kvchunk(wsrc, ci):
                    c0, c1 = ci * 1024, min((ci + 1) * 1024, D)
                    t = stream.tile([128, NSL, 1024], f8, tag="wkv", bufs=2,
                                    name="kvw")
                    nc.sync.dma_start(t[:, :, 0:c1 - c0], re3(wsrc[:, c0:c1]))
                    return t
                wkt = [kvchunk(wkr8, ci) for ci in range(3)]

                # filler: next segment q_r projection under the AR wait
                if s + 1 < NSEG:
                    phase1(s + 1, "q")

                attnS = wrk.tile([128, NT * 128], bf16, tag="abuf", bufs=2)
                nc.gpsimd.dma_start(attnS[:], b1_out[0:128, :])
                nc.gpsimd.dma_start(dbpack[32:33, :], b1_out[128:129, 0:H * 128])
                denswS = wrk.tile([1, H * 128], f32, tag="denswS")
                nc.vector.tensor_copy(denswS[:], dbpack[32:33, :])
                recw = wrk.tile([1, H * 128], f32, tag="recw")
                nc.vector.reciprocal(recw[:], denswS[:])  # = SA / densum
                attnN8 = wrk.tile([128, NT, 128], f8, tag="attnN8")
                rwrep = {}
                for h in range(H):
                    rwrep[h] = wrk.tile([128, 128], f32, tag="rwrep", bufs=2,
                                        name="rwrep_h")
                    nc.gpsimd.partition_broadcast(
                        rwrep[h][:], recw[:, h * 128:(h + 1) * 128])
                    for off, sz in _pieces(h * HD, HD):
                        g, o = off // 128, off % 128
                        nc.vector.tensor_mul(
                            attnN8[o:o + sz, g, :],
                            attnS[o:o + sz, g * 128:(g + 1) * 128],
                            rwrep[h][o:o + sz, :])

                # -- new_mem^T = wo_w.T @ attnN (replicated) --
                new_sb = wrk.tile([128, NT * 128], bf16, tag="new")
                for m0 in range(0, NT, 4):
                    wt = wtile()
                    nc.sync.dma_start(
                        wt[:], re3(wow8[:, m0 * 128:(m0 + 4) * 128]))
                    for mi in range(4):
                        m = m0 + mi
                        ps = ps256()
                        for j in range(NT // 2):
                            nc.tensor.matmul(
                                ps[:, 0:128],
                                wt[:, 2 * j:2 * j + 2,
                                   mi * 128:(mi + 1) * 128],
                                attnN8[:, 2 * j:2 * j + 2, :],
                                start=(j == 0), stop=(j == NT // 2 - 1),
                                perf_mode=DR)
                        nc.scalar.activation(
                            new_sb[:, m * 128:(m + 1) * 128], ps[:, 0:128],
                            AF.Copy, scale=1.0 / (SW * SA))
                newT8 = wrk.tile([128, NT, 128], f8, tag="newT8", bufs=2)
                nc.vector.tensor_scalar_mul(
                    newT8[:].rearrange("p j m -> p (j m)"), new_sb[:], SM)

                # -- memory update (own 640-slice) --
                new_sl = new_sb[:, bass.ds(toff, SL)]
                if s == 0:
                    msl = wrk.tile([128, SL], bf16, tag="mslbf", bufs=2)
                    nc.vector.tensor_copy(msl[:], new_sl)
                    mem_slbf = msl
                    memT8 = newT8
                else:
                    # wgw gate gemm: one host-prearranged [5120, 128]
                    # m-tile stream at a time; slabs 0..19 contract the
                    # memory half, 20..39 the new_mem half.
                    gsl = wrk.tile([128, SL], f32, tag="gsl")
                    for m in range(NSL):
                        wt = stream.tile([128, 40, 128], f8, tag="w8",
                                         bufs=3, name="wgwt")
                        nc.scalar.dma_start(
                            wt[:].rearrange("p j m -> p (j m)"),
                            wgw8[:, m * 5120:(m + 1) * 5120])
                        ps = ps256()
                        for half, rhs8 in ((0, memT8), (1, newT8)):
                            for j in range(NT // 2):
                                nc.tensor.matmul(
                                    ps[:, 0:128],
                                    wt[:, half * NT + 2 * j:
                                       half * NT + 2 * j + 2, :],
                                    rhs8[:, 2 * j:2 * j + 2, :],
                                    start=(half == 0 and j == 0),
                                    stop=(half == 1 and j == NT // 2 - 1),
                                    perf_mode=DR)
                        nc.scalar.activation(
                            gsl[:, m * 128:(m + 1) * 128], ps[:, 0:128],
                            AF.Sigmoid, bias=bgwt[:, m:m + 1],
                            scale=1.0 / (SW * SM))
                    dlt = wrk.tile([128, SL], f32, tag="dlt")
                    nc.vector.tensor_sub(dlt[:], new_sl, mem_slbf[:])
                    nc.vector.tensor_mul(dlt[:], gsl[:], dlt[:])
                    msl = wrk.tile([128, SL], bf16, tag="mslbf", bufs=2)
                    nc.vector.tensor_add(msl[:], mem_slbf[:], dlt[:])
                    mem_slbf = msl

                # -- k_r/v_r partials from the updated slice + AR#2 --
                memsl8 = wrk.tile([128, NSL, 128], f8, tag="memsl8")
                nc.vector.tensor_scalar_mul(
                    memsl8[:].rearrange("p j m -> p (j m)"), mem_slbf[:], SM)
                pk = wrk.tile([128, NT * 128], bf16, tag="abuf", bufs=2)
                for g in range(NT):
                    t = wkt[g // 8]
                    gc = (g % 8) * 128
                    ps = ps256()
                    for j in range(2):
                        nc.tensor.matmul(
                            ps[:, 0:128],
                            t[:, 2 * j:2 * j + 2, gc:gc + 128],
                            memsl8[:, 2 * j:2 * j + 2, :],
                            start=(j == 0), stop=False, perf_mode=DR)
                    nc.tensor.matmul(
                        ps[:, 0:128], t[:, 4, gc:gc + 128],
                        memsl8[:, 4, :], start=False, stop=True)
                    nc.scalar.copy(pk[:, g * 128:(g + 1) * 128], ps[:, 0:128])
                pv = wrk.tile([128, NT * 128], bf16, tag="abuf", bufs=2)
                wvt = [kvchunk(wvr8, ci) for ci in range(3)]
                for ch in range(5):
                    t = wvt[ch // 2]
                    cc = (ch % 2) * 512
                    ps = ps512()
                    for j in range(2):
                        nc.tensor.matmul(
                            ps, memsl8[:, 2 * j:2 * j + 2, :],
                            t[:, 2 * j:2 * j + 2, cc:cc + 512],
                            start=(j == 0), stop=False, perf_mode=DR)
                    nc.tensor.matmul(
                        ps, memsl8[:, 4, :],
                        t[:, 4, cc:cc + 512],
                        start=False, stop=True)
                    nc.scalar.copy(pv[:, ch * 512:(ch + 1) * 512], ps)
                # full mem_{s+1} is only needed (by segment s+1's wgw gemm)
                # when it isn't already replicated: at s=0 mem_1 = new_mem
                # (replicated), at s=2 mem_3 is never used beyond k_r/v_r.
                need_mem_bcast = s == 1
                bi, bo = (b2_in, b2_out) if need_mem_bcast else (b2s_in,
                                                                 b2s_out)
                if need_mem_bcast:
                    stagem = wrk.tile([128, NT * 128], bf16, tag="stage")
                    nc.vector.memset(stagem[:], 0.0)
                    nc.vector.tensor_copy(stagem[:, bass.ds(toff, SL)],
                                          mem_slbf[:])
                    nc.gpsimd.dma_start(bi[256:384, :], stagem[:])
                nc.gpsimd.dma_start(bi[0:128, :], pk[:])
                nc.gpsimd.dma_start(bi[128:256, :], pv[:])
                nc.gpsimd.collective_compute(
                    "AllReduce", mybir.AluOpType.add,
                    ins=[bi.opt()], outs=[bo.opt()],
                    replica_groups=GROUPS)

                # filler: next segment gate projection under the AR wait
                if s + 1 < NSEG:
                    phase1(s + 1, "g")

                nc.gpsimd.dma_start(krT[:], bo[0:128, :])
                nc.gpsimd.dma_start(vr[:], bo[128:256, :])
                if need_mem_bcast and s >= 1:
                    memTbf = wrk.tile([128, NT * 128], bf16, tag="stage")
                    nc.gpsimd.dma_start(memTbf[:], bo[256:384, :])
                    memT8 = wrk.tile([128, NT, 128], f8, tag="newT8", bufs=2)
                    nc.vector.tensor_scalar_mul(
                        memT8[:].rearrange("p j m -> p (j m)"), memTbf[:], SM)

    nc.compile()
    return nc


def _prep_inputs(inputs):
    hs = np.asarray(inputs["hidden_states"], np.float32)

    def f8w(x):
        return np.ascontiguousarray(
            (np.asarray(x, np.float32) * SW).astype(F8))

    wq_w = np.asarray(inputs["wq_w"], np.float32)
    wk_w = np.asarray(inputs["wk_w"], np.float32)
    wk_r = np.asarray(inputs["wk_r"], np.float32)
    wv_r = np.asarray(inputs["wv_r"], np.float32)
    wg_w = np.asarray(inputs["wg_w"], np.float32)
    im = np.asarray(inputs["initial_memory"], np.float32)[0]     # [M, D]
    mq = np.asarray(inputs["write_queries"], np.float32)[0]      # [M, D]

    qw = mq @ wq_w                                               # [M, D]
    P = np.empty((D, H * MEM), np.float32)
    for h in range(H):
        hsl = slice(h * HD, (h + 1) * HD)
        P[:, h * MEM:(h + 1) * MEM] = wk_w[:, hsl] @ qw[:, hsl].T
    P8 = np.ascontiguousarray((P * SP).astype(F8))

    kr0 = (im @ wk_r) * (SM * SW)   # scaled like the AR'd partials
    # slab-major k_r^T: krT[p, j*128+m] = kr0[m, j*128+p]
    kr0_sm = np.ascontiguousarray(
        kr0.T.reshape(NT, 128, MEM).transpose(1, 0, 2).reshape(128, NT * MEM)
        .astype(BF16))
    vr0 = np.ascontiguousarray(((im @ wv_r) * (SM * SW)).astype(BF16))

    bgr = np.asarray(inputs["bg_r"], np.float32)                 # [D]
    bgr_t = np.ascontiguousarray(bgr.reshape(NT, 128).T)         # [128, NT]
    bgw = np.asarray(inputs["bg_w"], np.float32)

    shared = {
        "wqr8": f8w(inputs["wq_r"]), "wgr8": f8w(inputs["wg_r"]),
        "wor8": f8w(inputs["wo_r"]), "wvw8": f8w(inputs["wv_w"]),
        "wow8": f8w(inputs["wo_w"]),
        "P8": P8, "kr0": kr0_sm, "vr0": vr0, "bgr": bgr_t,
    }

    in_maps = []
    for c in range(8):
        b, t = c // 4, c % 4
        cols = np.concatenate(
            [np.arange(s * 1024 + t * TOK, s * 1024 + (t + 1) * TOK)
             for s in range(NSEG)])
        xt = np.ascontiguousarray(hs[b, cols, :].T)              # [D, 1024]
        imap = dict(shared)
        imap["x8"] = np.ascontiguousarray((xt * SX).astype(F8))
        imap["xtb"] = np.ascontiguousarray(xt.astype(BF16))
        wgsl = (wg_w[:, t * SL:(t + 1) * SL] * SW).astype(F8)  # [2D, SL]
        # slab-major per m-tile: [128, (m, j, c)] with w[j*128+p, m*128+c]
        imap["wgw8"] = np.ascontiguousarray(
            wgsl.reshape(40, 128, NSL, 128).transpose(1, 2, 0, 3)
            .reshape(128, NSL * 40 * 128))
        imap["wkr8"] = f8w(wk_r[t * SL:(t + 1) * SL, :])
        imap["wvr8"] = f8w(wv_r[t * SL:(t + 1) * SL, :])
        imap["bgw"] = np.ascontiguousarray(
            bgw[t * SL:(t + 1) * SL].reshape(NSL, 128).T)        # [128, NSL]
        in_maps.append(imap)
    return in_maps


def _run(inputs, trace=False):
    from concourse.bass_utils import run_bass_kernel_spmd
    if "nc" not in _CACHE:
        _CACHE["nc"] = _build()
    in_maps = _prep_inputs(inputs)
    res = run_bass_kernel_spmd(_CACHE["nc"], in_maps, list(range(8)),
                               trace=trace)
    hs = np.asarray(inputs["hidden_states"])
    out = np.empty((hs.shape[0], NSEG * 1024, D), np.float32)
    for c in range(8):
        b, t = c // 4, c % 4
        htc = np.asarray(res.results[c]["ht"], np.float32)       # [D, 1024]
        for s in range(NSEG):
            out[b, s * 1024 + t * TOK:s * 1024 + (t + 1) * TOK, :] = \
                htc[:, s * TOK:(s + 1) * TOK].T
    return out, res


def kernel(**inputs):
    out, _ = _run(inputs, trace=False)
    return out
